# revision 34
# baseline (speedup 1.0000x reference)
"""AttLocRec (location-aware recurrent attention) Trainium2 kernel.

Data-parallel over batch: 8 cores x 4 batches/core, no collectives.
Per core:
  pre_T = (enc @ V_w)^T     PE fp32r, enc pre-transposed on host
  conv -> feat -> LSTM      tiny (PE + ACT + DVE), phase 0
  bias_e = h@W_w + z@U_w + b  PE, transposed to per-partition layout
  e_T   = tanh(pre_T + bias_e)   fused on ACT (per-partition bias)
  score = g^T e_T           PE (M=32 zero-padded), masked via host mask
  w     = softmax(2*score)  DVE max + ACT exp(accum) + normalize
  c_v   = enc_T @ w         gpsimd partition_broadcast + DVE
                            scalar_tensor_tensor (fused mul+reduce)
Weights/side inputs ride in one packed SBUF blob (single DMA).
"""

import os

import numpy as np

import concourse.bass as bass
import concourse.bacc as bacc
import concourse.tile as tile
from concourse import mybir
from concourse.bass_utils import run_bass_kernel_spmd

B, T, E, A, D = 32, 2048, 512, 512, 512
C, F = 32, 100
KF = 2 * F + 1  # 201
TPAD = T + 2 * F  # 2248
NCORES = 8
BPC = B // NCORES  # 4
SCALING = 2.0

F32 = mybir.dt.float32
F32R = mybir.dt.float32r
BF16 = mybir.dt.bfloat16
AF = mybir.ActivationFunctionType
ALU = mybir.AluOpType

DEBUG_SKIP = set(os.environ.get("KERNEL_DEBUG_SKIP", "").split(",")) - {""}
_STAGES = ["minimal", "phase0", "pre", "score_mm", "score_copy", "score_ttr",
           "score", "softmax", "cv_nobcast", "full"]
STAGE = _STAGES.index(os.environ.get("KERNEL_VARIANT", "full"))

# blob column offsets (fp32 elements per partition)
OFF_VW = 0           # (128, 2048): vw[kc*512 + n] = V_w[kc*128+p, n]
OFF_G = 2048         # (128, 128): g zero-padded, col ac*32 holds g chunk ac
OFF_ID = 2176        # (4, 4) identity
OFF_CW0 = 2180       # (128, 32) conv_wT rows 0..127
OFF_CW1 = 2212       # (73, 32)  conv_wT rows 128..200
OFF_AHT = 2244       # (128, 16): [p, kc*4+b] = att_hT[kc*128+p, b]
OFF_DZT = 2260       # (128, 16)
OFF_ATTC = 2276      # (4, 512) att_c
OFF_ONES = 2788      # (1, 4) ones
BLOB_COLS = 2792


def build_program():
    nc = bacc.Bacc(None, target_bir_lowering=False)

    # ---- per-core DRAM inputs ----
    enc_T = nc.dram_tensor("enc_T", [BPC, E, T], F32R, kind="ExternalInput")
    att_pad = nc.dram_tensor("att_pad", [BPC, TPAD], F32R, kind="ExternalInput")
    neg_mask = nc.dram_tensor("neg_mask", [BPC, T], F32, kind="ExternalInput")
    blob_d = nc.dram_tensor("blob", [128, BLOB_COLS], F32R, kind="ExternalInput")
    W_ih_aug = nc.dram_tensor("W_ih_aug", [C + 1, 4 * A], F32R, kind="ExternalInput")
    W_hhT = nc.dram_tensor("W_hhT", [A, 4 * A], F32R, kind="ExternalInput")
    W_stack = nc.dram_tensor("W_stack", [2 * D + 1, A], F32R, kind="ExternalInput")
    # ---- per-core outputs ----
    cv_out = nc.dram_tensor("cv_out", [BPC, E], F32, kind="ExternalOutput")
    w_out = nc.dram_tensor("w_out", [BPC, T], F32, kind="ExternalOutput")
    h_out = nc.dram_tensor("h_out", [BPC, A], F32, kind="ExternalOutput")
    c_out = nc.dram_tensor("c_out", [BPC, A], F32, kind="ExternalOutput")

    with tile.TileContext(nc) as tc:
        with (
            tc.tile_pool(name="wts", bufs=1) as wts,
            tc.tile_pool(name="enc", bufs=8) as encp,
            tc.tile_pool(name="et", bufs=5) as etp,
            tc.tile_pool(name="scr", bufs=1) as scrp,
            tc.tile_pool(name="rows", bufs=2) as rowp,
            tc.tile_pool(name="small", bufs=2) as smallp,
            tc.tile_pool(name="ps_pre", bufs=3, space="PSUM") as ps_pre,
            tc.tile_pool(name="ps_sc", bufs=4, space="PSUM") as ps_sc,
            tc.tile_pool(name="ps_p0", bufs=1, space="PSUM") as ps_p0,
            tc.tile_pool(name="p0", bufs=1) as p0p,
        ):
            # ---- resident blob: ONE dma ----
            blob = wts.tile([128, BLOB_COLS], F32R, tag="blob")
            nc.sync.dma_start(out=blob, in_=blob_d[:, :])
            vw = [blob[:, OFF_VW + kc * 512: OFF_VW + (kc + 1) * 512]
                  for kc in range(4)]
            g_sb = blob[:, OFF_G:OFF_G + 128]
            id_sb = blob[0:BPC, OFF_ID:OFF_ID + BPC].bitcast(F32)
            cw0 = blob[:, OFF_CW0:OFF_CW0 + C]
            cw1 = blob[0:KF - 128, OFF_CW1:OFF_CW1 + C]
            ahT = blob[:, OFF_AHT:OFF_AHT + 4 * BPC]
            dzT = blob[:, OFF_DZT:OFF_DZT + 4 * BPC]
            attc = blob[0:BPC, OFF_ATTC:OFF_ATTC + A].bitcast(F32)
            ones_r = blob[0:1, OFF_ONES:OFF_ONES + BPC]

            enc_tiles = {}

            def load_enc(b):
                # split each chunk into 4 sub-DMAs: each DMA's descriptors
                # serialize on one of 16 DMA engines (~19GB/s each), so
                # spreading quadruples effective enc bandwidth
                for ec in range(4):
                    t_ = encp.tile([128, T], F32R, tag="enc")
                    for s in range(4):
                        nc.sync.dma_start(
                            out=t_[s * 32:(s + 1) * 32, :],
                            in_=enc_T[b, ec * 128 + s * 32:
                                      ec * 128 + (s + 1) * 32, :])
                    enc_tiles[(b, ec)] = t_

            load_enc(0)

            # ============ phase 0: conv + LSTM + bias_e ============
            if STAGE >= 1:
                # conv: im2col per (batch, kchunk), full T width
                featT_aug = wts.tile([C + 1, BPC], F32R, tag="feat")
                nc.gpsimd.dma_start(out=featT_aug[C:C + 1, :], in_=ones_r)
                for b in range(BPC):
                    im0 = p0p.tile([128, T], F32R, tag="im0", bufs=1)
                    for s in range(4):
                        nc.sync.dma_start(
                            out=im0[s * 32:(s + 1) * 32, :],
                            in_=bass.AP(att_pad, b * TPAD + s * 32,
                                        [[1, 32], [1, T]]))
                    im1 = p0p.tile([128, T], F32R, tag="im1", bufs=1)
                    for s in range(3):
                        p0_, p1_ = s * 32, min((s + 1) * 32, KF - 128)
                        nc.sync.dma_start(
                            out=im1[p0_:p1_, :],
                            in_=bass.AP(att_pad, b * TPAD + 128 + p0_,
                                        [[1, p1_ - p0_], [1, T]]))
                    fmax = smallp.tile([C, 4], F32, tag="fmax")
                    for tcn in range(4):
                        pconv = ps_p0.tile([C, 512], F32, tag="p0")
                        nc.tensor.matmul(
                            pconv, cw0, im0[:, tcn * 512:(tcn + 1) * 512],
                            start=True, stop=False)
                        nc.tensor.matmul(
                            pconv, cw1, im1[:KF - 128, tcn * 512:(tcn + 1) * 512],
                            start=False, stop=True)
                        nc.vector.reduce_max(
                            out=fmax[:, tcn:tcn + 1], in_=pconv,
                            axis=mybir.AxisListType.X)
                    fm1 = smallp.tile([C, 1], F32, tag="fm1")
                    nc.vector.reduce_max(out=fm1, in_=fmax,
                                         axis=mybir.AxisListType.X)
                    nc.vector.tensor_scalar_max(
                        out=featT_aug[0:C, b:b + 1], in0=fm1, scalar1=0.0)

                # LSTM gates = [feat;1]@W_ih_aug + att_h@W_hhT
                gate_sb = []
                gate_funcs = [AF.Sigmoid, AF.Sigmoid, AF.Tanh, AF.Sigmoid]
                for tcn in range(4):
                    pg = ps_p0.tile([BPC, 512], F32, tag="p0")
                    wih_c = p0p.tile([C + 1, 512], F32R, tag="wih", bufs=2)
                    nc.gpsimd.dma_start(
                        out=wih_c, in_=W_ih_aug[:, tcn * 512:(tcn + 1) * 512])
                    nc.tensor.matmul(pg, featT_aug, wih_c,
                                     start=True, stop=False)
                    for kc in range(4):
                        whh_c = p0p.tile([128, 512], F32R, tag="whh", bufs=2)
                        nc.gpsimd.dma_start(
                            out=whh_c,
                            in_=W_hhT[kc * 128:(kc + 1) * 128,
                                      tcn * 512:(tcn + 1) * 512])
                        nc.tensor.matmul(
                            pg, ahT[:, kc * BPC:(kc + 1) * BPC], whh_c,
                            start=False, stop=(kc == 3))
                    gs = smallp.tile([BPC, 512], F32, tag="gate", bufs=4)
                    nc.scalar.activation(out=gs, in_=pg, func=gate_funcs[tcn])
                    gate_sb.append(gs)

                sig_i, sig_f, tanh_g, sig_o = gate_sb
                c_new = wts.tile([BPC, A], F32, tag="cnew")
                tmp = smallp.tile([BPC, A], F32, tag="ctmp")
                nc.vector.tensor_mul(out=c_new, in0=sig_f, in1=attc)
                nc.vector.tensor_mul(out=tmp, in0=sig_i, in1=tanh_g)
                nc.vector.tensor_add(out=c_new, in0=c_new, in1=tmp)
                tanh_c = smallp.tile([BPC, A], F32, tag="tanhc")
                nc.scalar.activation(out=tanh_c, in_=c_new, func=AF.Tanh)
                h_new = wts.tile([BPC, A], F32, tag="hnew")
                nc.vector.tensor_mul(out=h_new, in0=sig_o, in1=tanh_c)
                nc.sync.dma_start(out=c_out[:, :], in_=c_new)
                nc.sync.dma_start(out=h_out[:, :], in_=h_new)

                # h_new^T via PE transpose
                hT_sb = wts.tile([128, 4 * BPC], F32R, tag="hT")
                for ec in range(4):
                    ptr = ps_p0.tile([128, BPC], F32, tag="p0")
                    nc.tensor.transpose(
                        ptr, h_new[:, ec * 128:(ec + 1) * 128], id_sb)
                    nc.scalar.copy(
                        out=hT_sb[:, ec * BPC:(ec + 1) * BPC], in_=ptr)

                # bias_e = h@W_w + z@U_w + (W_b+U_b+V_b)
                pbias = ps_p0.tile([BPC, A], F32, tag="p0")
                for kc in range(9):
                    if kc < 4:
                        lhsT = hT_sb[:, kc * BPC:(kc + 1) * BPC]
                    elif kc < 8:
                        lhsT = dzT[:, (kc - 4) * BPC:(kc - 3) * BPC]
                    else:
                        lhsT = ones_r
                    p = min(128, 2 * D + 1 - kc * 128)
                    ws_c = p0p.tile([128, A], F32R, tag="wstack", bufs=2)
                    nc.gpsimd.dma_start(
                        out=ws_c[:p, :], in_=W_stack[kc * 128:kc * 128 + p, :])
                    nc.tensor.matmul(pbias, lhsT, ws_c[:p, :],
                                     start=(kc == 0), stop=(kc == 8))
                bias_sb = smallp.tile([BPC, A], F32, tag="biasrow")
                nc.scalar.copy(out=bias_sb, in_=pbias)
                biasT_sb = wts.tile([128, 4 * BPC], F32, tag="biasT")
                for ac in range(4):
                    ptr = ps_p0.tile([128, BPC], F32, tag="p0")
                    nc.tensor.transpose(
                        ptr, bias_sb[:, ac * 128:(ac + 1) * 128], id_sb)
                    nc.scalar.copy(
                        out=biasT_sb[:, ac * BPC:(ac + 1) * BPC], in_=ptr)

            # ============ main loop over batches ============
            wb_sb = scrp.tile([128, T], F32, tag="wbcast")
            cv_scr = scrp.tile([128, T], F32, tag="cvscr")
            for b in range(BPC):
                if b + 1 < BPC:
                    load_enc(b + 1)
                if STAGE < 2:
                    continue
                # pre_T + fused tanh(+bias) -> e_T, with the score
                # matmuls interleaved per-ac so they overlap pre instead
                # of forming a serial tail phase
                do_score = STAGE >= 3
                if do_score:
                    mask_row = rowp.tile([1, T], F32, tag="maskrow", bufs=1)
                    nc.gpsimd.dma_start(out=mask_row,
                                        in_=neg_mask[b:b + 1, :])
                    psc_l = []
                    for _t in range(4):
                        psc = ps_sc.tile([32, 512], F32, tag="sc")
                        psc_l.append(psc)
                et_tiles = []
                for ac in range(4):
                    et = etp.tile([128, T], F32R, tag="et")
                    for tcn in range(4):
                        pp = ps_pre.tile([128, 512], F32, tag="pre")
                        for kc in range(4):
                            nc.tensor.matmul(
                                pp,
                                vw[kc][:, ac * 128:(ac + 1) * 128],
                                enc_tiles[(b, kc)][:, tcn * 512:(tcn + 1) * 512],
                                start=(kc == 0), stop=(kc == 3))
                        nc.scalar.activation(
                            out=et[:, tcn * 512:(tcn + 1) * 512], in_=pp,
                            func=AF.Tanh,
                            bias=biasT_sb[:, ac * BPC + b:ac * BPC + b + 1])
                    if do_score:
                        for tcn in range(4):
                            nc.tensor.matmul(
                                psc_l[tcn], g_sb[:, ac * 32:(ac + 1) * 32],
                                et[:, tcn * 512:(tcn + 1) * 512],
                                start=(ac == 0), stop=(ac == 3))
                    et_tiles.append(et)
                if STAGE < 4:
                    continue
                score_sb = rowp.tile([1, T], F32, tag="score")
                mx4 = smallp.tile([1, 4], F32, tag="mx4")
                for tcn in range(4):
                    nc.vector.tensor_add(
                        out=score_sb[:, tcn * 512:(tcn + 1) * 512],
                        in0=psc_l[tcn][0:1, :],
                        in1=mask_row[:, tcn * 512:(tcn + 1) * 512])
                    nc.vector.reduce_max(
                        out=mx4[:, tcn:tcn + 1],
                        in_=score_sb[:, tcn * 512:(tcn + 1) * 512],
                        axis=mybir.AxisListType.X)
                if STAGE < 6:
                    continue
                mx = smallp.tile([1, 1], F32, tag="mx")
                nc.vector.reduce_max(out=mx, in_=mx4,
                                     axis=mybir.AxisListType.X)
                if STAGE < 7:
                    continue
                neg2m = smallp.tile([1, 1], F32, tag="neg2m")
                nc.scalar.mul(out=neg2m, in_=mx, mul=-SCALING)
                ssum = smallp.tile([1, 1], F32, tag="ssum")
                nc.scalar.activation(
                    out=score_sb, in_=score_sb, func=AF.Exp,
                    bias=neg2m, scale=SCALING, accum_out=ssum)
                # c_v with UNNORMALIZED weights; rescale at the end.
                if STAGE >= 8:
                    if "bcast" in DEBUG_SKIP or STAGE < 9:
                        nc.vector.memset(wb_sb, 0.0)
                    else:
                        nc.gpsimd.partition_broadcast(wb_sb, score_sb)
                rsum = smallp.tile([1, 1], F32, tag="rsum")
                nc.vector.reciprocal(out=rsum, in_=ssum)
                nc.scalar.mul(out=score_sb, in_=score_sb, mul=rsum)
                nc.sync.dma_start(out=w_out[b:b + 1, :], in_=score_sb)
                if STAGE < 8:
                    continue
                rbc = smallp.tile([128, 1], F32, tag="rbc")
                nc.gpsimd.partition_broadcast(rbc, rsum)
                cv_sb = smallp.tile([128, 4], F32, tag="cv")
                for ec in range(4):
                    nc.vector.scalar_tensor_tensor(
                        out=cv_scr, in0=enc_tiles[(b, ec)].bitcast(F32),
                        scalar=1.0, in1=wb_sb,
                        op0=ALU.mult, op1=ALU.mult,
                        accum_out=cv_sb[:, ec:ec + 1])
                nc.vector.tensor_scalar_mul(out=cv_sb, in0=cv_sb, scalar1=rbc)
                nc.sync.dma_start(
                    out=bass.AP(cv_out, b * E, [[1, 128], [128, 4]]),
                    in_=cv_sb)
    nc.compile()
    return nc


_program = None
_last_in_maps = None


def _get_program():
    global _program
    if _program is None:
        _program = build_program()
    return _program


def _make_blob(V_w, g_w, conv_wT, att_hT_s, dec_zT_s, att_c_s):
    blob = np.zeros((128, BLOB_COLS), dtype=np.float32)
    for kc in range(4):
        blob[:, OFF_VW + kc * 512: OFF_VW + (kc + 1) * 512] = \
            V_w[kc * 128:(kc + 1) * 128, :]
        blob[:, OFF_G + kc * 32] = g_w[kc * 128:(kc + 1) * 128, 0]
        blob[:, OFF_AHT + kc * BPC: OFF_AHT + (kc + 1) * BPC] = \
            att_hT_s[kc * 128:(kc + 1) * 128, :]
        blob[:, OFF_DZT + kc * BPC: OFF_DZT + (kc + 1) * BPC] = \
            dec_zT_s[kc * 128:(kc + 1) * 128, :]
    blob[0:BPC, OFF_ID:OFF_ID + BPC] = np.eye(BPC, dtype=np.float32)
    blob[:, OFF_CW0:OFF_CW0 + C] = conv_wT[0:128, :]
    blob[0:KF - 128, OFF_CW1:OFF_CW1 + C] = conv_wT[128:KF, :]
    blob[0:BPC, OFF_ATTC:OFF_ATTC + A] = att_c_s
    blob[0:1, OFF_ONES:OFF_ONES + BPC] = 1.0
    return blob


def prep_in_maps(inputs):
    enc = np.ascontiguousarray(inputs["enc_hs_pad"], dtype=np.float32)
    enc_len = np.asarray(inputs["enc_hs_len"])
    dec_z = np.ascontiguousarray(inputs["dec_z"], dtype=np.float32)
    att_prev = np.ascontiguousarray(inputs["att_prev"], dtype=np.float32)
    att_h = np.ascontiguousarray(inputs["att_h"], dtype=np.float32)
    att_c = np.ascontiguousarray(inputs["att_c"], dtype=np.float32)
    W_w, W_b = np.asarray(inputs["W_w"]), np.asarray(inputs["W_b"])
    V_w, V_b = np.asarray(inputs["V_w"]), np.asarray(inputs["V_b"])
    U_w, U_b = np.asarray(inputs["U_w"]), np.asarray(inputs["U_b"])
    g_w = np.asarray(inputs["g_w"])
    conv_w = np.asarray(inputs["conv_w"])
    W_ih, W_hh = np.asarray(inputs["W_ih"]), np.asarray(inputs["W_hh"])
    b_ih, b_hh = np.asarray(inputs["b_ih"]), np.asarray(inputs["b_hh"])

    enc_T = np.ascontiguousarray(enc.transpose(0, 2, 1))  # (B, E, T)
    att_pad = np.zeros((B, TPAD), dtype=np.float32)
    att_pad[:, F:F + T] = att_prev
    neg_mask = np.where(
        np.arange(T)[None, :] >= enc_len[:, None],
        np.float32(-1e30), np.float32(0.0)).astype(np.float32)
    dec_zT = np.ascontiguousarray(dec_z.T)
    att_hT = np.ascontiguousarray(att_h.T)
    conv_wT = np.ascontiguousarray(conv_w[:, 0, :].T).astype(np.float32)
    W_ih_aug = np.concatenate(
        [W_ih.T, (b_ih + b_hh)[None, :]], axis=0).astype(np.float32)
    W_hhT = np.ascontiguousarray(W_hh.T).astype(np.float32)
    W_stack = np.concatenate(
        [W_w, U_w, (W_b + U_b + V_b)[None, :]], axis=0).astype(np.float32)

    in_maps = []
    for c in range(NCORES):
        s = slice(c * BPC, (c + 1) * BPC)
        blob = _make_blob(V_w.astype(np.float32), g_w.astype(np.float32),
                          conv_wT, att_hT[:, s], dec_zT[:, s], att_c[s])
        in_maps.append({
            "enc_T": np.ascontiguousarray(enc_T[s]),
            "att_pad": np.ascontiguousarray(att_pad[s]),
            "neg_mask": np.ascontiguousarray(neg_mask[s]),
            "blob": blob,
            "W_ih_aug": W_ih_aug,
            "W_hhT": W_hhT,
            "W_stack": W_stack,
        })
    return in_maps


def kernel(**inputs):
    nc = _get_program()
    in_maps = prep_in_maps(inputs)
    global _last_in_maps
    _last_in_maps = in_maps
    res = run_bass_kernel_spmd(nc, in_maps, core_ids=list(range(NCORES)))
    results = res.results

    c_v = np.concatenate([r["cv_out"] for r in results], axis=0)
    w = np.concatenate([r["w_out"] for r in results], axis=0)
    h_new = np.concatenate([r["h_out"] for r in results], axis=0)
    c_new = np.concatenate([r["c_out"] for r in results], axis=0)
    return c_v, w, h_new, c_new


# revision 35
# speedup vs baseline: 1.0634x; 1.0634x over previous
"""AttLocRec (location-aware recurrent attention) Trainium2 kernel.

Data-parallel over batch: 8 cores x 4 batches/core, no collectives.
Per core:
  pre_T = (enc @ V_w)^T     PE fp32r, enc pre-transposed on host
  conv -> feat -> LSTM      tiny (PE + ACT + DVE), phase 0
  bias_e = h@W_w + z@U_w + b  PE, transposed to per-partition layout
  e_T   = tanh(pre_T + bias_e)   fused on ACT (per-partition bias)
  score = g^T e_T           PE (M=32 zero-padded), masked via host mask
  w     = softmax(2*score)  DVE max + ACT exp(accum) + normalize
  c_v   = enc_T @ w         gpsimd partition_broadcast + DVE
                            scalar_tensor_tensor (fused mul+reduce)
Weights/side inputs ride in one packed SBUF blob (single DMA).
"""

import os

import numpy as np

import concourse.bass as bass
import concourse.bacc as bacc
import concourse.tile as tile
from concourse import mybir
from concourse.bass_utils import run_bass_kernel_spmd

B, T, E, A, D = 32, 2048, 512, 512, 512
C, F = 32, 100
KF = 2 * F + 1  # 201
TPAD = T + 2 * F  # 2248
NCORES = 8
BPC = B // NCORES  # 4
SCALING = 2.0

F32 = mybir.dt.float32
F32R = mybir.dt.float32r
BF16 = mybir.dt.bfloat16
AF = mybir.ActivationFunctionType
ALU = mybir.AluOpType

DEBUG_SKIP = set(os.environ.get("KERNEL_DEBUG_SKIP", "").split(",")) - {""}
_STAGES = ["minimal", "phase0", "pre", "score_mm", "score_copy", "score_ttr",
           "score", "softmax", "cv_nobcast", "full"]
STAGE = _STAGES.index(os.environ.get("KERNEL_VARIANT", "full"))

# blob column offsets (fp32 elements per partition)
OFF_VW = 0           # (128, 2048): vw[kc*512 + n] = V_w[kc*128+p, n]
OFF_G = 2048         # (128, 128): g zero-padded, col ac*32 holds g chunk ac
OFF_ID = 2176        # (4, 4) identity
OFF_CW0 = 2180       # (128, 32) conv_wT rows 0..127
OFF_CW1 = 2212       # (73, 32)  conv_wT rows 128..200
OFF_AHT = 2244       # (128, 16): [p, kc*4+b] = att_hT[kc*128+p, b]
OFF_DZT = 2260       # (128, 16)
OFF_ATTC = 2276      # (4, 512) att_c
OFF_ONES = 2788      # (1, 4) ones
BLOB_COLS = 2792


def build_program():
    nc = bacc.Bacc(None, target_bir_lowering=False)

    # ---- per-core DRAM inputs ----
    enc_T = nc.dram_tensor("enc_T", [BPC, E, T], F32R, kind="ExternalInput")
    att_pad = nc.dram_tensor("att_pad", [BPC, TPAD], F32R, kind="ExternalInput")
    neg_mask = nc.dram_tensor("neg_mask", [BPC, T], F32, kind="ExternalInput")
    blob_d = nc.dram_tensor("blob", [128, BLOB_COLS], F32R, kind="ExternalInput")
    W_ih_aug = nc.dram_tensor("W_ih_aug", [C + 1, 4 * A], F32R, kind="ExternalInput")
    W_hhT = nc.dram_tensor("W_hhT", [A, 4 * A], F32R, kind="ExternalInput")
    W_stack = nc.dram_tensor("W_stack", [2 * D + 1, A], F32R, kind="ExternalInput")
    # ---- per-core outputs ----
    cv_out = nc.dram_tensor("cv_out", [BPC, E], F32, kind="ExternalOutput")
    w_out = nc.dram_tensor("w_out", [BPC, T], F32, kind="ExternalOutput")
    h_out = nc.dram_tensor("h_out", [BPC, A], F32, kind="ExternalOutput")
    c_out = nc.dram_tensor("c_out", [BPC, A], F32, kind="ExternalOutput")

    with tile.TileContext(nc) as tc:
        with (
            tc.tile_pool(name="wts", bufs=1) as wts,
            tc.tile_pool(name="enc", bufs=8) as encp,
            tc.tile_pool(name="et", bufs=5) as etp,
            tc.tile_pool(name="scr", bufs=1) as scrp,
            tc.tile_pool(name="rows", bufs=2) as rowp,
            tc.tile_pool(name="small", bufs=2) as smallp,
            tc.tile_pool(name="ps_pre", bufs=4, space="PSUM") as ps_pre,
            tc.tile_pool(name="ps_sc", bufs=2, space="PSUM") as ps_sc,
            tc.tile_pool(name="ps_p0", bufs=2, space="PSUM") as ps_p0,
            tc.tile_pool(name="p0", bufs=1) as p0p,
        ):
            # ---- resident blob: ONE dma ----
            blob = wts.tile([128, BLOB_COLS], F32R, tag="blob")
            nc.sync.dma_start(out=blob, in_=blob_d[:, :])
            vw = [blob[:, OFF_VW + kc * 512: OFF_VW + (kc + 1) * 512]
                  for kc in range(4)]
            g_sb = blob[:, OFF_G:OFF_G + 128]
            id_sb = blob[0:BPC, OFF_ID:OFF_ID + BPC].bitcast(F32)
            cw0 = blob[:, OFF_CW0:OFF_CW0 + C]
            cw1 = blob[0:KF - 128, OFF_CW1:OFF_CW1 + C]
            ahT = blob[:, OFF_AHT:OFF_AHT + 4 * BPC]
            dzT = blob[:, OFF_DZT:OFF_DZT + 4 * BPC]
            attc = blob[0:BPC, OFF_ATTC:OFF_ATTC + A].bitcast(F32)
            ones_r = blob[0:1, OFF_ONES:OFF_ONES + BPC]

            enc_tiles = {}

            def load_enc(b):
                # split each chunk into 4 sub-DMAs: each DMA's descriptors
                # serialize on one of 16 DMA engines (~19GB/s each), so
                # spreading quadruples effective enc bandwidth
                for ec in range(4):
                    t_ = encp.tile([128, T], F32R, tag="enc")
                    for s in range(4):
                        nc.sync.dma_start(
                            out=t_[s * 32:(s + 1) * 32, :],
                            in_=enc_T[b, ec * 128 + s * 32:
                                      ec * 128 + (s + 1) * 32, :])
                    enc_tiles[(b, ec)] = t_

            load_enc(0)

            # ============ phase 0: conv + LSTM + bias_e ============
            if STAGE >= 1:
                # conv: im2col per (batch, kchunk), full T width
                featT_aug = wts.tile([C + 1, BPC], F32R, tag="feat")
                nc.gpsimd.dma_start(out=featT_aug[C:C + 1, :], in_=ones_r)
                for b in range(BPC):
                    im0 = p0p.tile([128, T], F32R, tag="im0", bufs=1)
                    for s in range(4):
                        nc.sync.dma_start(
                            out=im0[s * 32:(s + 1) * 32, :],
                            in_=bass.AP(att_pad, b * TPAD + s * 32,
                                        [[1, 32], [1, T]]))
                    im1 = p0p.tile([128, T], F32R, tag="im1", bufs=1)
                    for s in range(3):
                        p0_, p1_ = s * 32, min((s + 1) * 32, KF - 128)
                        nc.sync.dma_start(
                            out=im1[p0_:p1_, :],
                            in_=bass.AP(att_pad, b * TPAD + 128 + p0_,
                                        [[1, p1_ - p0_], [1, T]]))
                    fmax = smallp.tile([C, 4], F32, tag="fmax")
                    for tcn in range(4):
                        pconv = ps_p0.tile([C, 512], F32, tag="p0")
                        nc.tensor.matmul(
                            pconv, cw0, im0[:, tcn * 512:(tcn + 1) * 512],
                            start=True, stop=False)
                        nc.tensor.matmul(
                            pconv, cw1, im1[:KF - 128, tcn * 512:(tcn + 1) * 512],
                            start=False, stop=True)
                        nc.vector.reduce_max(
                            out=fmax[:, tcn:tcn + 1], in_=pconv,
                            axis=mybir.AxisListType.X)
                    fm1 = smallp.tile([C, 1], F32, tag="fm1")
                    nc.vector.reduce_max(out=fm1, in_=fmax,
                                         axis=mybir.AxisListType.X)
                    nc.vector.tensor_scalar_max(
                        out=featT_aug[0:C, b:b + 1], in0=fm1, scalar1=0.0)

                # LSTM gates = [feat;1]@W_ih_aug + att_h@W_hhT
                gate_sb = []
                gate_funcs = [AF.Sigmoid, AF.Sigmoid, AF.Tanh, AF.Sigmoid]
                for tcn in range(4):
                    pg = ps_p0.tile([BPC, 512], F32, tag="p0")
                    wih_c = p0p.tile([C + 1, 512], F32R, tag="wih", bufs=2)
                    nc.gpsimd.dma_start(
                        out=wih_c, in_=W_ih_aug[:, tcn * 512:(tcn + 1) * 512])
                    nc.tensor.matmul(pg, featT_aug, wih_c,
                                     start=True, stop=False)
                    for kc in range(4):
                        whh_c = p0p.tile([128, 512], F32R, tag="whh", bufs=2)
                        nc.gpsimd.dma_start(
                            out=whh_c,
                            in_=W_hhT[kc * 128:(kc + 1) * 128,
                                      tcn * 512:(tcn + 1) * 512])
                        nc.tensor.matmul(
                            pg, ahT[:, kc * BPC:(kc + 1) * BPC], whh_c,
                            start=False, stop=(kc == 3))
                    gs = smallp.tile([BPC, 512], F32, tag="gate", bufs=4)
                    nc.scalar.activation(out=gs, in_=pg, func=gate_funcs[tcn])
                    gate_sb.append(gs)

                sig_i, sig_f, tanh_g, sig_o = gate_sb
                c_new = wts.tile([BPC, A], F32, tag="cnew")
                tmp = smallp.tile([BPC, A], F32, tag="ctmp")
                nc.vector.tensor_mul(out=c_new, in0=sig_f, in1=attc)
                nc.vector.tensor_mul(out=tmp, in0=sig_i, in1=tanh_g)
                nc.vector.tensor_add(out=c_new, in0=c_new, in1=tmp)
                tanh_c = smallp.tile([BPC, A], F32, tag="tanhc")
                nc.scalar.activation(out=tanh_c, in_=c_new, func=AF.Tanh)
                h_new = wts.tile([BPC, A], F32, tag="hnew")
                nc.vector.tensor_mul(out=h_new, in0=sig_o, in1=tanh_c)
                nc.sync.dma_start(out=c_out[:, :], in_=c_new)
                nc.sync.dma_start(out=h_out[:, :], in_=h_new)

                # h_new^T via PE transpose
                hT_sb = wts.tile([128, 4 * BPC], F32R, tag="hT")
                for ec in range(4):
                    ptr = ps_p0.tile([128, BPC], F32, tag="p0")
                    nc.tensor.transpose(
                        ptr, h_new[:, ec * 128:(ec + 1) * 128], id_sb)
                    nc.scalar.copy(
                        out=hT_sb[:, ec * BPC:(ec + 1) * BPC], in_=ptr)

                # bias_e = h@W_w + z@U_w + (W_b+U_b+V_b)
                pbias = ps_p0.tile([BPC, A], F32, tag="p0")
                for kc in range(9):
                    if kc < 4:
                        lhsT = hT_sb[:, kc * BPC:(kc + 1) * BPC]
                    elif kc < 8:
                        lhsT = dzT[:, (kc - 4) * BPC:(kc - 3) * BPC]
                    else:
                        lhsT = ones_r
                    p = min(128, 2 * D + 1 - kc * 128)
                    ws_c = p0p.tile([128, A], F32R, tag="wstack", bufs=2)
                    nc.gpsimd.dma_start(
                        out=ws_c[:p, :], in_=W_stack[kc * 128:kc * 128 + p, :])
                    nc.tensor.matmul(pbias, lhsT, ws_c[:p, :],
                                     start=(kc == 0), stop=(kc == 8))
                bias_sb = smallp.tile([BPC, A], F32, tag="biasrow")
                nc.scalar.copy(out=bias_sb, in_=pbias)
                biasT_sb = wts.tile([128, 4 * BPC], F32, tag="biasT")
                for ac in range(4):
                    ptr = ps_p0.tile([128, BPC], F32, tag="p0")
                    nc.tensor.transpose(
                        ptr, bias_sb[:, ac * 128:(ac + 1) * 128], id_sb)
                    nc.scalar.copy(
                        out=biasT_sb[:, ac * BPC:(ac + 1) * BPC], in_=ptr)

            # ============ main loop over batches ============
            wb_sb = scrp.tile([128, T], F32, tag="wbcast")
            cv_scr = scrp.tile([128, T], F32, tag="cvscr")
            for b in range(BPC):
                if b + 1 < BPC:
                    load_enc(b + 1)
                if STAGE < 2:
                    continue
                # pre_T + fused tanh(+bias) -> e_T
                et_tiles = []
                for ac in range(4):
                    et = etp.tile([128, T], F32R, tag="et")
                    for tcn in range(4):
                        pp = ps_pre.tile([128, 512], F32, tag="pre")
                        for kc in range(4):
                            nc.tensor.matmul(
                                pp,
                                vw[kc][:, ac * 128:(ac + 1) * 128],
                                enc_tiles[(b, kc)][:, tcn * 512:(tcn + 1) * 512],
                                start=(kc == 0), stop=(kc == 3))
                        nc.scalar.activation(
                            out=et[:, tcn * 512:(tcn + 1) * 512], in_=pp,
                            func=AF.Tanh,
                            bias=biasT_sb[:, ac * BPC + b:ac * BPC + b + 1])
                    et_tiles.append(et)
                # score = g^T e_T (+ mask, running max)
                if STAGE < 3:
                    continue
                mask_row = rowp.tile([1, T], F32, tag="maskrow", bufs=1)
                nc.gpsimd.dma_start(out=mask_row, in_=neg_mask[b:b + 1, :])
                score_sb = rowp.tile([1, T], F32, tag="score")
                mx4 = smallp.tile([1, 4], F32, tag="mx4")
                for tcn in range(4):
                    psc = ps_sc.tile([32, 512], F32, tag="sc")
                    for ac in range(4):
                        nc.tensor.matmul(
                            psc, g_sb[:, ac * 32:(ac + 1) * 32],
                            et_tiles[ac][:, tcn * 512:(tcn + 1) * 512],
                            start=(ac == 0), stop=(ac == 3))
                    if STAGE == 3:
                        continue
                    nc.vector.tensor_add(
                        out=score_sb[:, tcn * 512:(tcn + 1) * 512],
                        in0=psc[0:1, :],
                        in1=mask_row[:, tcn * 512:(tcn + 1) * 512])
                    nc.vector.reduce_max(
                        out=mx4[:, tcn:tcn + 1],
                        in_=score_sb[:, tcn * 512:(tcn + 1) * 512],
                        axis=mybir.AxisListType.X)
                if STAGE < 6:
                    continue
                mx = smallp.tile([1, 1], F32, tag="mx")
                nc.vector.reduce_max(out=mx, in_=mx4,
                                     axis=mybir.AxisListType.X)
                if STAGE < 7:
                    continue
                neg2m = smallp.tile([1, 1], F32, tag="neg2m")
                nc.scalar.mul(out=neg2m, in_=mx, mul=-SCALING)
                ssum = smallp.tile([1, 1], F32, tag="ssum")
                nc.scalar.activation(
                    out=score_sb, in_=score_sb, func=AF.Exp,
                    bias=neg2m, scale=SCALING, accum_out=ssum)
                # c_v with UNNORMALIZED weights; rescale at the end.
                if STAGE >= 8:
                    if "bcast" in DEBUG_SKIP or STAGE < 9:
                        nc.vector.memset(wb_sb, 0.0)
                    else:
                        nc.gpsimd.partition_broadcast(wb_sb, score_sb)
                rsum = smallp.tile([1, 1], F32, tag="rsum")
                nc.vector.reciprocal(out=rsum, in_=ssum)
                nc.scalar.mul(out=score_sb, in_=score_sb, mul=rsum)
                nc.sync.dma_start(out=w_out[b:b + 1, :], in_=score_sb)
                if STAGE < 8:
                    continue
                rbc = smallp.tile([128, 1], F32, tag="rbc")
                nc.gpsimd.partition_broadcast(rbc, rsum)
                cv_sb = smallp.tile([128, 4], F32, tag="cv")
                for ec in range(4):
                    nc.vector.scalar_tensor_tensor(
                        out=cv_scr, in0=enc_tiles[(b, ec)].bitcast(F32),
                        scalar=1.0, in1=wb_sb,
                        op0=ALU.mult, op1=ALU.mult,
                        accum_out=cv_sb[:, ec:ec + 1])
                nc.vector.tensor_scalar_mul(out=cv_sb, in0=cv_sb, scalar1=rbc)
                nc.sync.dma_start(
                    out=bass.AP(cv_out, b * E, [[1, 128], [128, 4]]),
                    in_=cv_sb)
    nc.compile()
    return nc


_program = None
_last_in_maps = None


def _get_program():
    global _program
    if _program is None:
        _program = build_program()
    return _program


def _make_blob(V_w, g_w, conv_wT, att_hT_s, dec_zT_s, att_c_s):
    blob = np.zeros((128, BLOB_COLS), dtype=np.float32)
    for kc in range(4):
        blob[:, OFF_VW + kc * 512: OFF_VW + (kc + 1) * 512] = \
            V_w[kc * 128:(kc + 1) * 128, :]
        blob[:, OFF_G + kc * 32] = g_w[kc * 128:(kc + 1) * 128, 0]
        blob[:, OFF_AHT + kc * BPC: OFF_AHT + (kc + 1) * BPC] = \
            att_hT_s[kc * 128:(kc + 1) * 128, :]
        blob[:, OFF_DZT + kc * BPC: OFF_DZT + (kc + 1) * BPC] = \
            dec_zT_s[kc * 128:(kc + 1) * 128, :]
    blob[0:BPC, OFF_ID:OFF_ID + BPC] = np.eye(BPC, dtype=np.float32)
    blob[:, OFF_CW0:OFF_CW0 + C] = conv_wT[0:128, :]
    blob[0:KF - 128, OFF_CW1:OFF_CW1 + C] = conv_wT[128:KF, :]
    blob[0:BPC, OFF_ATTC:OFF_ATTC + A] = att_c_s
    blob[0:1, OFF_ONES:OFF_ONES + BPC] = 1.0
    return blob


def prep_in_maps(inputs):
    enc = np.ascontiguousarray(inputs["enc_hs_pad"], dtype=np.float32)
    enc_len = np.asarray(inputs["enc_hs_len"])
    dec_z = np.ascontiguousarray(inputs["dec_z"], dtype=np.float32)
    att_prev = np.ascontiguousarray(inputs["att_prev"], dtype=np.float32)
    att_h = np.ascontiguousarray(inputs["att_h"], dtype=np.float32)
    att_c = np.ascontiguousarray(inputs["att_c"], dtype=np.float32)
    W_w, W_b = np.asarray(inputs["W_w"]), np.asarray(inputs["W_b"])
    V_w, V_b = np.asarray(inputs["V_w"]), np.asarray(inputs["V_b"])
    U_w, U_b = np.asarray(inputs["U_w"]), np.asarray(inputs["U_b"])
    g_w = np.asarray(inputs["g_w"])
    conv_w = np.asarray(inputs["conv_w"])
    W_ih, W_hh = np.asarray(inputs["W_ih"]), np.asarray(inputs["W_hh"])
    b_ih, b_hh = np.asarray(inputs["b_ih"]), np.asarray(inputs["b_hh"])

    enc_T = np.ascontiguousarray(enc.transpose(0, 2, 1))  # (B, E, T)
    att_pad = np.zeros((B, TPAD), dtype=np.float32)
    att_pad[:, F:F + T] = att_prev
    neg_mask = np.where(
        np.arange(T)[None, :] >= enc_len[:, None],
        np.float32(-1e30), np.float32(0.0)).astype(np.float32)
    dec_zT = np.ascontiguousarray(dec_z.T)
    att_hT = np.ascontiguousarray(att_h.T)
    conv_wT = np.ascontiguousarray(conv_w[:, 0, :].T).astype(np.float32)
    W_ih_aug = np.concatenate(
        [W_ih.T, (b_ih + b_hh)[None, :]], axis=0).astype(np.float32)
    W_hhT = np.ascontiguousarray(W_hh.T).astype(np.float32)
    W_stack = np.concatenate(
        [W_w, U_w, (W_b + U_b + V_b)[None, :]], axis=0).astype(np.float32)

    in_maps = []
    for c in range(NCORES):
        s = slice(c * BPC, (c + 1) * BPC)
        blob = _make_blob(V_w.astype(np.float32), g_w.astype(np.float32),
                          conv_wT, att_hT[:, s], dec_zT[:, s], att_c[s])
        in_maps.append({
            "enc_T": np.ascontiguousarray(enc_T[s]),
            "att_pad": np.ascontiguousarray(att_pad[s]),
            "neg_mask": np.ascontiguousarray(neg_mask[s]),
            "blob": blob,
            "W_ih_aug": W_ih_aug,
            "W_hhT": W_hhT,
            "W_stack": W_stack,
        })
    return in_maps


def kernel(**inputs):
    nc = _get_program()
    in_maps = prep_in_maps(inputs)
    global _last_in_maps
    _last_in_maps = in_maps
    res = run_bass_kernel_spmd(nc, in_maps, core_ids=list(range(NCORES)))
    results = res.results

    c_v = np.concatenate([r["cv_out"] for r in results], axis=0)
    w = np.concatenate([r["w_out"] for r in results], axis=0)
    h_new = np.concatenate([r["h_out"] for r in results], axis=0)
    c_new = np.concatenate([r["c_out"] for r in results], axis=0)
    return c_v, w, h_new, c_new


# revision 36
# speedup vs baseline: 1.0871x; 1.0223x over previous
"""AttLocRec (location-aware recurrent attention) Trainium2 kernel.

Data-parallel over batch: 8 cores x 4 batches/core, no collectives.
Per core:
  pre_T = (enc @ V_w)^T     PE fp32r, enc pre-transposed on host
  conv -> feat -> LSTM      tiny (PE + ACT + DVE), phase 0
  bias_e = h@W_w + z@U_w + b  PE, transposed to per-partition layout
  e_T   = tanh(pre_T + bias_e)   fused on ACT (per-partition bias)
  score = g^T e_T           PE (M=32 zero-padded), masked via host mask
  w     = softmax(2*score)  DVE max + ACT exp(accum) + normalize
  c_v   = enc_T @ w         gpsimd partition_broadcast + DVE
                            scalar_tensor_tensor (fused mul+reduce)
Weights/side inputs ride in one packed SBUF blob (single DMA).
"""

import os

import numpy as np

import concourse.bass as bass
import concourse.bacc as bacc
import concourse.tile as tile
from concourse import mybir
from concourse.bass_utils import run_bass_kernel_spmd

B, T, E, A, D = 32, 2048, 512, 512, 512
C, F = 32, 100
KF = 2 * F + 1  # 201
TPAD = T + 2 * F  # 2248
NCORES = 8
BPC = B // NCORES  # 4
SCALING = 2.0

F32 = mybir.dt.float32
F32R = mybir.dt.float32r
BF16 = mybir.dt.bfloat16
AF = mybir.ActivationFunctionType
ALU = mybir.AluOpType

DEBUG_SKIP = set(os.environ.get("KERNEL_DEBUG_SKIP", "").split(",")) - {""}
_STAGES = ["minimal", "phase0", "pre", "score_mm", "score_copy", "score_ttr",
           "score", "softmax", "cv_nobcast", "full"]
STAGE = _STAGES.index(os.environ.get("KERNEL_VARIANT", "full"))

# blob column offsets (fp32 elements per partition)
OFF_VW = 0           # (128, 2048): vw[kc*512 + n] = V_w[kc*128+p, n]
OFF_G = 2048         # (128, 128): g zero-padded, col ac*32 holds g chunk ac
OFF_ID = 2176        # (4, 4) identity
OFF_CW0 = 2180       # (128, 32) conv_wT rows 0..127
OFF_CW1 = 2212       # (73, 32)  conv_wT rows 128..200
OFF_AHT = 2244       # (128, 16): [p, kc*4+b] = att_hT[kc*128+p, b]
OFF_DZT = 2260       # (128, 16)
OFF_ATTC = 2276      # (4, 512) att_c
OFF_ONES = 2788      # (1, 4) ones
BLOB_COLS = 2792


def build_program():
    nc = bacc.Bacc(None, target_bir_lowering=False)

    # ---- per-core DRAM inputs ----
    enc_T = nc.dram_tensor("enc_T", [BPC, E, T], F32R, kind="ExternalInput")
    att_pad = nc.dram_tensor("att_pad", [BPC, TPAD], F32R, kind="ExternalInput")
    neg_mask = nc.dram_tensor("neg_mask", [BPC, T], F32, kind="ExternalInput")
    blob_d = nc.dram_tensor("blob", [128, BLOB_COLS], F32R, kind="ExternalInput")
    W_ih_aug = nc.dram_tensor("W_ih_aug", [C + 1, 4 * A], F32R, kind="ExternalInput")
    W_hhT = nc.dram_tensor("W_hhT", [A, 4 * A], F32R, kind="ExternalInput")
    W_stack = nc.dram_tensor("W_stack", [2 * D + 1, A], F32R, kind="ExternalInput")
    # ---- per-core outputs ----
    cv_out = nc.dram_tensor("cv_out", [BPC, E], F32, kind="ExternalOutput")
    w_out = nc.dram_tensor("w_out", [BPC, T], F32, kind="ExternalOutput")
    h_out = nc.dram_tensor("h_out", [BPC, A], F32, kind="ExternalOutput")
    c_out = nc.dram_tensor("c_out", [BPC, A], F32, kind="ExternalOutput")

    with tile.TileContext(nc) as tc:
        with (
            tc.tile_pool(name="wts", bufs=1) as wts,
            tc.tile_pool(name="enc", bufs=8) as encp,
            tc.tile_pool(name="et", bufs=5) as etp,
            tc.tile_pool(name="scr", bufs=1) as scrp,
            tc.tile_pool(name="rows", bufs=2) as rowp,
            tc.tile_pool(name="small", bufs=2) as smallp,
            tc.tile_pool(name="ps_pre", bufs=4, space="PSUM") as ps_pre,
            tc.tile_pool(name="ps_sc", bufs=2, space="PSUM") as ps_sc,
            tc.tile_pool(name="ps_p0", bufs=2, space="PSUM") as ps_p0,
            tc.tile_pool(name="p0", bufs=1) as p0p,
        ):
            # ---- resident blob: 4 sub-DMAs (spread across DMA engines) ----
            blob = wts.tile([128, BLOB_COLS], F32R, tag="blob")
            for s in range(4):
                nc.sync.dma_start(out=blob[s * 32:(s + 1) * 32, :],
                                  in_=blob_d[s * 32:(s + 1) * 32, :])
            vw = [blob[:, OFF_VW + kc * 512: OFF_VW + (kc + 1) * 512]
                  for kc in range(4)]
            g_sb = blob[:, OFF_G:OFF_G + 128]
            id_sb = blob[0:BPC, OFF_ID:OFF_ID + BPC].bitcast(F32)
            cw0 = blob[:, OFF_CW0:OFF_CW0 + C]
            cw1 = blob[0:KF - 128, OFF_CW1:OFF_CW1 + C]
            ahT = blob[:, OFF_AHT:OFF_AHT + 4 * BPC]
            dzT = blob[:, OFF_DZT:OFF_DZT + 4 * BPC]
            attc = blob[0:BPC, OFF_ATTC:OFF_ATTC + A].bitcast(F32)
            ones_r = blob[0:1, OFF_ONES:OFF_ONES + BPC]

            enc_tiles = {}

            def load_enc(b):
                # split each chunk into 4 sub-DMAs: each DMA's descriptors
                # serialize on one of 16 DMA engines (~19GB/s each), so
                # spreading quadruples effective enc bandwidth
                for ec in range(4):
                    t_ = encp.tile([128, T], F32R, tag="enc")
                    for s in range(4):
                        nc.sync.dma_start(
                            out=t_[s * 32:(s + 1) * 32, :],
                            in_=enc_T[b, ec * 128 + s * 32:
                                      ec * 128 + (s + 1) * 32, :])
                    enc_tiles[(b, ec)] = t_

            load_enc(0)

            # ============ phase 0: conv + LSTM + bias_e ============
            if STAGE >= 1:
                # conv: im2col per (batch, kchunk), full T width
                featT_aug = wts.tile([C + 1, BPC], F32R, tag="feat")
                nc.gpsimd.dma_start(out=featT_aug[C:C + 1, :], in_=ones_r)
                for b in range(BPC):
                    im0 = p0p.tile([128, T], F32R, tag="im0", bufs=1)
                    for s in range(4):
                        nc.scalar.dma_start(
                            out=im0[s * 32:(s + 1) * 32, :],
                            in_=bass.AP(att_pad, b * TPAD + s * 32,
                                        [[1, 32], [1, T]]))
                    im1 = p0p.tile([128, T], F32R, tag="im1", bufs=1)
                    for s in range(3):
                        p0_, p1_ = s * 32, min((s + 1) * 32, KF - 128)
                        nc.scalar.dma_start(
                            out=im1[p0_:p1_, :],
                            in_=bass.AP(att_pad, b * TPAD + 128 + p0_,
                                        [[1, p1_ - p0_], [1, T]]))
                    fmax = smallp.tile([C, 4], F32, tag="fmax")
                    for tcn in range(4):
                        pconv = ps_p0.tile([C, 512], F32, tag="p0")
                        nc.tensor.matmul(
                            pconv, cw0, im0[:, tcn * 512:(tcn + 1) * 512],
                            start=True, stop=False)
                        nc.tensor.matmul(
                            pconv, cw1, im1[:KF - 128, tcn * 512:(tcn + 1) * 512],
                            start=False, stop=True)
                        nc.vector.reduce_max(
                            out=fmax[:, tcn:tcn + 1], in_=pconv,
                            axis=mybir.AxisListType.X)
                    fm1 = smallp.tile([C, 1], F32, tag="fm1")
                    nc.vector.reduce_max(out=fm1, in_=fmax,
                                         axis=mybir.AxisListType.X)
                    nc.vector.tensor_scalar_max(
                        out=featT_aug[0:C, b:b + 1], in0=fm1, scalar1=0.0)

                # LSTM gates = [feat;1]@W_ih_aug + att_h@W_hhT
                gate_sb = []
                gate_funcs = [AF.Sigmoid, AF.Sigmoid, AF.Tanh, AF.Sigmoid]
                for tcn in range(4):
                    pg = ps_p0.tile([BPC, 512], F32, tag="p0")
                    wih_c = p0p.tile([C + 1, 512], F32R, tag="wih", bufs=2)
                    nc.gpsimd.dma_start(
                        out=wih_c, in_=W_ih_aug[:, tcn * 512:(tcn + 1) * 512])
                    nc.tensor.matmul(pg, featT_aug, wih_c,
                                     start=True, stop=False)
                    for kc in range(4):
                        whh_c = p0p.tile([128, 512], F32R, tag="whh", bufs=2)
                        nc.gpsimd.dma_start(
                            out=whh_c,
                            in_=W_hhT[kc * 128:(kc + 1) * 128,
                                      tcn * 512:(tcn + 1) * 512])
                        nc.tensor.matmul(
                            pg, ahT[:, kc * BPC:(kc + 1) * BPC], whh_c,
                            start=False, stop=(kc == 3))
                    gs = smallp.tile([BPC, 512], F32, tag="gate", bufs=4)
                    nc.scalar.activation(out=gs, in_=pg, func=gate_funcs[tcn])
                    gate_sb.append(gs)

                sig_i, sig_f, tanh_g, sig_o = gate_sb
                c_new = wts.tile([BPC, A], F32, tag="cnew")
                tmp = smallp.tile([BPC, A], F32, tag="ctmp")
                nc.vector.tensor_mul(out=c_new, in0=sig_f, in1=attc)
                nc.vector.tensor_mul(out=tmp, in0=sig_i, in1=tanh_g)
                nc.vector.tensor_add(out=c_new, in0=c_new, in1=tmp)
                tanh_c = smallp.tile([BPC, A], F32, tag="tanhc")
                nc.scalar.activation(out=tanh_c, in_=c_new, func=AF.Tanh)
                h_new = wts.tile([BPC, A], F32, tag="hnew")
                nc.vector.tensor_mul(out=h_new, in0=sig_o, in1=tanh_c)
                nc.sync.dma_start(out=c_out[:, :], in_=c_new)
                nc.sync.dma_start(out=h_out[:, :], in_=h_new)

                # h_new^T via PE transpose
                hT_sb = wts.tile([128, 4 * BPC], F32R, tag="hT")
                for ec in range(4):
                    ptr = ps_p0.tile([128, BPC], F32, tag="p0")
                    nc.tensor.transpose(
                        ptr, h_new[:, ec * 128:(ec + 1) * 128], id_sb)
                    nc.scalar.copy(
                        out=hT_sb[:, ec * BPC:(ec + 1) * BPC], in_=ptr)

                # bias_e = h@W_w + z@U_w + (W_b+U_b+V_b)
                pbias = ps_p0.tile([BPC, A], F32, tag="p0")
                for kc in range(9):
                    if kc < 4:
                        lhsT = hT_sb[:, kc * BPC:(kc + 1) * BPC]
                    elif kc < 8:
                        lhsT = dzT[:, (kc - 4) * BPC:(kc - 3) * BPC]
                    else:
                        lhsT = ones_r
                    p = min(128, 2 * D + 1 - kc * 128)
                    ws_c = p0p.tile([128, A], F32R, tag="wstack", bufs=2)
                    nc.gpsimd.dma_start(
                        out=ws_c[:p, :], in_=W_stack[kc * 128:kc * 128 + p, :])
                    nc.tensor.matmul(pbias, lhsT, ws_c[:p, :],
                                     start=(kc == 0), stop=(kc == 8))
                bias_sb = smallp.tile([BPC, A], F32, tag="biasrow")
                nc.scalar.copy(out=bias_sb, in_=pbias)
                biasT_sb = wts.tile([128, 4 * BPC], F32, tag="biasT")
                for ac in range(4):
                    ptr = ps_p0.tile([128, BPC], F32, tag="p0")
                    nc.tensor.transpose(
                        ptr, bias_sb[:, ac * 128:(ac + 1) * 128], id_sb)
                    nc.scalar.copy(
                        out=biasT_sb[:, ac * BPC:(ac + 1) * BPC], in_=ptr)

            # ============ main loop over batches ============
            wb_sb = scrp.tile([128, T], F32, tag="wbcast")
            cv_scr = scrp.tile([128, T], F32, tag="cvscr")
            for b in range(BPC):
                if b + 1 < BPC:
                    load_enc(b + 1)
                if STAGE < 2:
                    continue
                # pre_T + fused tanh(+bias) -> e_T
                et_tiles = []
                for ac in range(4):
                    et = etp.tile([128, T], F32R, tag="et")
                    for tcn in range(4):
                        pp = ps_pre.tile([128, 512], F32, tag="pre")
                        for kc in range(4):
                            nc.tensor.matmul(
                                pp,
                                vw[kc][:, ac * 128:(ac + 1) * 128],
                                enc_tiles[(b, kc)][:, tcn * 512:(tcn + 1) * 512],
                                start=(kc == 0), stop=(kc == 3))
                        nc.scalar.activation(
                            out=et[:, tcn * 512:(tcn + 1) * 512], in_=pp,
                            func=AF.Tanh,
                            bias=biasT_sb[:, ac * BPC + b:ac * BPC + b + 1])
                    et_tiles.append(et)
                # score = g^T e_T (+ mask, running max)
                if STAGE < 3:
                    continue
                mask_row = rowp.tile([1, T], F32, tag="maskrow", bufs=1)
                nc.gpsimd.dma_start(out=mask_row, in_=neg_mask[b:b + 1, :])
                score_sb = rowp.tile([1, T], F32, tag="score")
                mx4 = smallp.tile([1, 4], F32, tag="mx4")
                for tcn in range(4):
                    psc = ps_sc.tile([32, 512], F32, tag="sc")
                    for ac in range(4):
                        nc.tensor.matmul(
                            psc, g_sb[:, ac * 32:(ac + 1) * 32],
                            et_tiles[ac][:, tcn * 512:(tcn + 1) * 512],
                            start=(ac == 0), stop=(ac == 3))
                    if STAGE == 3:
                        continue
                    nc.vector.tensor_add(
                        out=score_sb[:, tcn * 512:(tcn + 1) * 512],
                        in0=psc[0:1, :],
                        in1=mask_row[:, tcn * 512:(tcn + 1) * 512])
                    nc.vector.reduce_max(
                        out=mx4[:, tcn:tcn + 1],
                        in_=score_sb[:, tcn * 512:(tcn + 1) * 512],
                        axis=mybir.AxisListType.X)
                if STAGE < 6:
                    continue
                mx = smallp.tile([1, 1], F32, tag="mx")
                nc.vector.reduce_max(out=mx, in_=mx4,
                                     axis=mybir.AxisListType.X)
                if STAGE < 7:
                    continue
                neg2m = smallp.tile([1, 1], F32, tag="neg2m")
                nc.scalar.mul(out=neg2m, in_=mx, mul=-SCALING)
                ssum = smallp.tile([1, 1], F32, tag="ssum")
                nc.scalar.activation(
                    out=score_sb, in_=score_sb, func=AF.Exp,
                    bias=neg2m, scale=SCALING, accum_out=ssum)
                # c_v with UNNORMALIZED weights; rescale at the end.
                if STAGE >= 8:
                    if "bcast" in DEBUG_SKIP or STAGE < 9:
                        nc.vector.memset(wb_sb, 0.0)
                    else:
                        nc.gpsimd.partition_broadcast(wb_sb, score_sb)
                rsum = smallp.tile([1, 1], F32, tag="rsum")
                nc.vector.reciprocal(out=rsum, in_=ssum)
                nc.scalar.mul(out=score_sb, in_=score_sb, mul=rsum)
                nc.sync.dma_start(out=w_out[b:b + 1, :], in_=score_sb)
                if STAGE < 8:
                    continue
                rbc = smallp.tile([128, 1], F32, tag="rbc")
                nc.gpsimd.partition_broadcast(rbc, rsum)
                cv_sb = smallp.tile([128, 4], F32, tag="cv")
                for ec in range(4):
                    nc.vector.scalar_tensor_tensor(
                        out=cv_scr, in0=enc_tiles[(b, ec)].bitcast(F32),
                        scalar=1.0, in1=wb_sb,
                        op0=ALU.mult, op1=ALU.mult,
                        accum_out=cv_sb[:, ec:ec + 1])
                nc.vector.tensor_scalar_mul(out=cv_sb, in0=cv_sb, scalar1=rbc)
                nc.sync.dma_start(
                    out=bass.AP(cv_out, b * E, [[1, 128], [128, 4]]),
                    in_=cv_sb)
    nc.compile()
    return nc


_program = None
_last_in_maps = None


def _get_program():
    global _program
    if _program is None:
        _program = build_program()
    return _program


def _make_blob(V_w, g_w, conv_wT, att_hT_s, dec_zT_s, att_c_s):
    blob = np.zeros((128, BLOB_COLS), dtype=np.float32)
    for kc in range(4):
        blob[:, OFF_VW + kc * 512: OFF_VW + (kc + 1) * 512] = \
            V_w[kc * 128:(kc + 1) * 128, :]
        blob[:, OFF_G + kc * 32] = g_w[kc * 128:(kc + 1) * 128, 0]
        blob[:, OFF_AHT + kc * BPC: OFF_AHT + (kc + 1) * BPC] = \
            att_hT_s[kc * 128:(kc + 1) * 128, :]
        blob[:, OFF_DZT + kc * BPC: OFF_DZT + (kc + 1) * BPC] = \
            dec_zT_s[kc * 128:(kc + 1) * 128, :]
    blob[0:BPC, OFF_ID:OFF_ID + BPC] = np.eye(BPC, dtype=np.float32)
    blob[:, OFF_CW0:OFF_CW0 + C] = conv_wT[0:128, :]
    blob[0:KF - 128, OFF_CW1:OFF_CW1 + C] = conv_wT[128:KF, :]
    blob[0:BPC, OFF_ATTC:OFF_ATTC + A] = att_c_s
    blob[0:1, OFF_ONES:OFF_ONES + BPC] = 1.0
    return blob


def prep_in_maps(inputs):
    enc = np.ascontiguousarray(inputs["enc_hs_pad"], dtype=np.float32)
    enc_len = np.asarray(inputs["enc_hs_len"])
    dec_z = np.ascontiguousarray(inputs["dec_z"], dtype=np.float32)
    att_prev = np.ascontiguousarray(inputs["att_prev"], dtype=np.float32)
    att_h = np.ascontiguousarray(inputs["att_h"], dtype=np.float32)
    att_c = np.ascontiguousarray(inputs["att_c"], dtype=np.float32)
    W_w, W_b = np.asarray(inputs["W_w"]), np.asarray(inputs["W_b"])
    V_w, V_b = np.asarray(inputs["V_w"]), np.asarray(inputs["V_b"])
    U_w, U_b = np.asarray(inputs["U_w"]), np.asarray(inputs["U_b"])
    g_w = np.asarray(inputs["g_w"])
    conv_w = np.asarray(inputs["conv_w"])
    W_ih, W_hh = np.asarray(inputs["W_ih"]), np.asarray(inputs["W_hh"])
    b_ih, b_hh = np.asarray(inputs["b_ih"]), np.asarray(inputs["b_hh"])

    enc_T = np.ascontiguousarray(enc.transpose(0, 2, 1))  # (B, E, T)
    att_pad = np.zeros((B, TPAD), dtype=np.float32)
    att_pad[:, F:F + T] = att_prev
    neg_mask = np.where(
        np.arange(T)[None, :] >= enc_len[:, None],
        np.float32(-1e30), np.float32(0.0)).astype(np.float32)
    dec_zT = np.ascontiguousarray(dec_z.T)
    att_hT = np.ascontiguousarray(att_h.T)
    conv_wT = np.ascontiguousarray(conv_w[:, 0, :].T).astype(np.float32)
    W_ih_aug = np.concatenate(
        [W_ih.T, (b_ih + b_hh)[None, :]], axis=0).astype(np.float32)
    W_hhT = np.ascontiguousarray(W_hh.T).astype(np.float32)
    W_stack = np.concatenate(
        [W_w, U_w, (W_b + U_b + V_b)[None, :]], axis=0).astype(np.float32)

    in_maps = []
    for c in range(NCORES):
        s = slice(c * BPC, (c + 1) * BPC)
        blob = _make_blob(V_w.astype(np.float32), g_w.astype(np.float32),
                          conv_wT, att_hT[:, s], dec_zT[:, s], att_c[s])
        in_maps.append({
            "enc_T": np.ascontiguousarray(enc_T[s]),
            "att_pad": np.ascontiguousarray(att_pad[s]),
            "neg_mask": np.ascontiguousarray(neg_mask[s]),
            "blob": blob,
            "W_ih_aug": W_ih_aug,
            "W_hhT": W_hhT,
            "W_stack": W_stack,
        })
    return in_maps


def kernel(**inputs):
    nc = _get_program()
    in_maps = prep_in_maps(inputs)
    global _last_in_maps
    _last_in_maps = in_maps
    res = run_bass_kernel_spmd(nc, in_maps, core_ids=list(range(NCORES)))
    results = res.results

    c_v = np.concatenate([r["cv_out"] for r in results], axis=0)
    w = np.concatenate([r["w_out"] for r in results], axis=0)
    h_new = np.concatenate([r["h_out"] for r in results], axis=0)
    c_new = np.concatenate([r["c_out"] for r in results], axis=0)
    return c_v, w, h_new, c_new
